# revision 68
# baseline (speedup 1.0000x reference)
"""Causal attention with L2-normalized Q/K — Trainium2 Bass kernel.

Problem shapes (hardcoded): X [2, 2048, 1024], Wq/Wk/Wv [1024, 1024],
Wo [1024, 1024], bo [1024]; H=16 heads, d_head=64.

Sharding: 8 cores = 2 batches x 4 head-groups (4 heads each).
Core c handles batch b=c//4, heads 4*(c%4)..4*(c%4)+3.
Each core computes QKV projections for its head slice, per-head
normalized causal attention, and a partial output projection
V_hat @ Wo[slice]. The partials are summed with per-q-chunk
ReduceScatters across the 4 cores of the batch (pipelined with
compute); the host reassembles the row strips.

Dtype/layout notes (error budget 2e-2; this lands ~4e-3):
- Q/K projections run in fp8e4 with DoubleRow perf mode: lhsT/rhs
  carry 2 stacked 128-row k-subtiles per instruction, so one matmul
  contracts 256 of d_model at ~265ns vs f32r's 331ns per 128 — 2.5x.
  Host pre-scales Wq/Wk by 16 to center them in e4m3's range (half of
  N(0,1/32) weights would be subnormal); the scale cancels exactly in
  the L2 normalization. HW-verified vs numpy at ~1e-4.
- V projection runs from host-cast bf16 X/Wv (halves its load bytes).
- X is shipped twice: fp8 [dr, qc, 128, 2, 512] DoubleRow tiles (2MB)
  for Q/K and bf16 [qc, i, 128, 512] (4MB) for V — 6MB vs 8MB f32,
  in 9 consolidated DMAs (the old per-tile loads made startup
  DMA-ISSUE bound at ~570ns/instr on the sync engine).
- Attention (scores S^T [k, q], exp on ACT, AV with a ones column
  accumulating the softmax denominator in row 64) stays float32r end
  to end. CAUTION: bf16 tiles written by DVE/ACT (not DMA) read back
  correctly via DMA but feed LDWEIGHTS garbage — bf16 qt/kt/pt/vhat
  stationaries all corrupted attention on HW (h01=1 worst) while
  passing CoreSim; f32r is also immune to the 130-byte (odd-element)
  stationary base offsets that come with the 65-wide V blocks.
  Also: a PSUM accumulation group must not contain matmuls writing
  different column sub-ranges (sliced-AV corrupted on HW), and
  mid-group dtype switches on the PE are suspect.
- Both norm halves' sum-of-squares come from ONE matmul against a
  [128, 2] block-ones stationary (rows 0/1 of one bank), so a single
  partition-parallel [2, 512] Ln/Exp pair computes 1/sqrt on ACT.
- Norm/denominator reciprocals stay in ACT's Exp/Ln table set
  (exp(-0.5 ln x), exp(-ln x)) and are broadcast across partitions
  via DRAM round trips on the scalar queue (DMA from DRAM may read
  with partition-stride 0).
- Output projection: vhat tiles and Wo in f32r (1 cy/row vs fp32's
  4), partials + per-chunk ReduceScatter + final output in bf16
  (host casts back to f32) — halves the collective bytes and the
  unoverlapped last-chunk RS tail.
- The 1e-6 in the reference's q/(|q|+eps) is dropped: |q| ~ 8, so the
  relative effect is ~1e-7, below fp32 noise.

Measured on trn2 (8 cores): 397us HW exec, rel err 4.2e-3
(baseline f32r version: 524us, 1.8e-4).
"""

import math
import numpy as np
import ml_dtypes
from contextlib import ExitStack

import concourse.bass as bass
import concourse.tile as tile
from concourse import mybir
from concourse.bass import _add_dep_helper as add_dep
from concourse.bass_utils import run_bass_kernel_spmd

F32 = mybir.dt.float32
F32R = mybir.dt.float32r  # TF32-like single-pass matmul dtype (~2e-4 rel)
BF16 = mybir.dt.bfloat16
FP8 = mybir.dt.float8e4
DR = mybir.MatmulPerfMode.DoubleRow
AF = mybir.ActivationFunctionType

B, N, D, H, DH = 2, 2048, 1024, 16, 64
NH = 4            # heads per core
J = NH * DH       # head dims per core = 256
P = 128
NQ = 512          # q chunk (moving free dim / psum bank)
NKT = N // P      # 16 k-tiles per head
ID = D // P       # 8 i-tiles of d_model
NDR = ID // 2     # 4 DoubleRow pairs of d_model
WSCALE = 16.0     # host pre-scale on Wq/Wk (cancels in L2 norm)
VW = DH + 1       # 65: V columns + ones column
NQC = N // NQ     # 4 q-chunks

_MAX_WAITS = 1


def _split_excess_waits(nc, limit=_MAX_WAITS):
    """This walrus build allows very few sem waits per instruction.
    Tile can emit many (kernel-tail Drain, collectives reading
    many-writer DRAM). Move excess waits onto injected same-engine
    NoOps right before the instruction; in-order execution preserves
    the semantics."""
    ctr = 0
    for fn in nc.m.functions:
        for bb in fn.blocks:
            out = []
            changed = False
            for ins in bb.instructions:
                si = ins.sync_info
                waits = list(si.on_wait) if si and si.on_wait else []
                if len(waits) > limit:
                    changed = True
                    chunks = [
                        waits[i : i + limit] for i in range(0, len(waits), limit)
                    ]
                    for ch in chunks[:-1]:
                        nop = mybir.InstNoOp(
                            name=f"I-waitsplit-{ctr}", ins=[], outs=[]
                        )
                        ctr += 1
                        nop.engine = ins.engine
                        nop.sync_info = mybir.SyncInfo(on_wait=ch, on_update=[])
                        out.append(nop)
                    ins.sync_info = mybir.SyncInfo(
                        on_wait=chunks[-1], on_update=list(si.on_update or [])
                    )
                out.append(ins)
            if changed:
                bb.instructions = out


def _build():
    nc = bass.Bass("TRN2", target_bir_lowering=False, debug=False, num_devices=8)

    x8 = nc.dram_tensor("x8", [NDR, NQC, P, 2, NQ], FP8, kind="ExternalInput").ap()
    xb = nc.dram_tensor("xb", [NQC, ID, P, NQ], BF16, kind="ExternalInput").ap()
    wq8 = nc.dram_tensor("wq8", [NDR, P, 2, J], FP8, kind="ExternalInput").ap()
    wk8 = nc.dram_tensor("wk8", [NDR, P, 2, J], FP8, kind="ExternalInput").ap()
    wvb = nc.dram_tensor("wvb", [ID, P, J], BF16, kind="ExternalInput").ap()
    wo = nc.dram_tensor("wo", [P, 2, D], F32R, kind="ExternalInput").ap()
    ones2d = nc.dram_tensor("ones2d", [P, 2], BF16, kind="ExternalInput").ap()
    bias4 = nc.dram_tensor("bias4", [D], F32, kind="ExternalInput").ap()
    maskd = nc.dram_tensor("maskd", [P, P], F32R, kind="ExternalInput").ap()
    onesd = nc.dram_tensor("onesd", [P, 1], F32R, kind="ExternalInput").ap()
    # output: 4 strips of [128, D]; strip qc = rows 512*qc + 128*rank
    # of this batch's final output (host reassembles)
    y_ext = nc.dram_tensor(
        "y", [NQC, 2, P, NQ], BF16, kind="ExternalOutput"
    ).ap()

    # per-q-chunk partial/reduced buffers so each ReduceScatter only
    # depends on its own chunk's stores
    # block layouts [t4, mc, p, 512] so every store/copy is one
    # contiguous descriptor; ReduceScatter scatters along dim0 (t4)
    yparts = [
        nc.dram_tensor(f"ypart{qc}", [4, 2, P, NQ], BF16) for qc in range(NQC)
    ]
    yrss = [nc.dram_tensor(f"yrs{qc}", [2, P, NQ], BF16) for qc in range(NQC)]
    # DRAM scratch rows for partition-broadcast round-trips
    nrow_d = nc.dram_tensor("nrow_d", [16, 2, NQ], F32)
    den_d = nc.dram_tensor("den_d", [16, NQ], F32)

    with tile.TileContext(nc) as tc:
        with ExitStack() as ctx:
            sb = ctx.enter_context(tc.tile_pool(name="sb", bufs=1))
            ps = ctx.enter_context(tc.tile_pool(name="ps", bufs=1, space="PSUM"))

            # ---- loads (few, large, consolidated) ----
            wq8_sb = sb.tile([P, NDR, 2, J], FP8, tag="wq8")
            nc.sync.dma_start(wq8_sb[:], wq8.rearrange("r p t j -> p r t j"))
            x8_sb = {}
            xb_sb = {}

            def load_x8(qc):
                t = sb.tile([P, NDR, 2, NQ], FP8, tag=f"x8q{qc}")
                nc.sync.dma_start(
                    t[:], x8[:, qc].rearrange("r p t n -> p r t n")
                )
                x8_sb[qc] = t

            def load_xbq(qc):
                t = sb.tile([P, ID, NQ], BF16, tag=f"xbq{qc}")
                nc.sync.dma_start(t[:], xb[qc].rearrange("i p n -> p i n"))
                xb_sb[qc] = t

            load_x8(0)
            wk8_sb = sb.tile([P, NDR, 2, J], FP8, tag="wk8")
            nc.sync.dma_start(wk8_sb[:], wk8.rearrange("r p t j -> p r t j"))
            ones2_sb = sb.tile([P, 2], BF16, tag="ones2")
            nc.sync.dma_start(ones2_sb[:], ones2d)
            wvb_sb = sb.tile([P, ID, J], BF16, tag="wvb")
            nc.sync.dma_start(wvb_sb[:], wvb.rearrange("i p j -> p i j"))
            load_xbq(0)
            for qc in range(1, NQC):
                load_x8(qc)
                load_xbq(qc)

            wo_sb = sb.tile([P, 2, D], F32R, tag="wo")
            nc.sync.dma_start(wo_sb[:], wo)
            bias_sb = sb.tile([P, D], F32, tag="bias")
            nc.sync.dma_start(
                bias_sb[:], bias4.rearrange("(a m) -> a m", a=1).to_broadcast((P, D))
            )
            mask_sb = sb.tile([P, P], F32R, tag="mask")
            nc.sync.dma_start(mask_sb[:], maskd)
            ones_sb = sb.tile([P, 1], F32R, tag="ones")
            nc.sync.dma_start(ones_sb[:], onesd)

            # ---- projections, quarter-major so compute tracks arrival ----
            qt_t = {}
            kt_t = {}
            v_sb = sb.tile([P, NKT, NH * VW], F32R, tag="v")
            v4 = v_sb.rearrange("p t (h x) -> p t h x", h=NH)
            # ones columns via broadcast-DMA (memset rejects f32r tiles)
            nc.sync.dma_start(
                v_sb.rearrange("p t (h x) -> p (t h) x", h=NH)[:, :, DH : DH + 1],
                onesd.rearrange("p (a b) -> p a b", a=1).to_broadcast(
                    (P, NKT * NH, 1)
                ),
            )

            def proj_chunk(w_sb, jt, tc4, out_t, ridx):
                # fp8 DoubleRow projection chunk + fused L2-norm scale
                pp = ps.tile([P, NQ], F32, tag="big", bufs=5)
                for r in range(NDR):
                    nc.tensor.matmul(
                        pp[:],
                        lhsT=w_sb[:, r, :, bass.ts(jt, P)],
                        rhs=x8_sb[tc4][:, r],
                        start=(r == 0),
                        stop=(r == NDR - 1),
                        perf_mode=DR,
                    )
                sq = sb.tile([P, NQ], BF16, tag="sq", bufs=3)
                nc.scalar.square(sq[:], pp[:])
                # one block-ones matmul -> both head-halves' row sums
                su = ps.tile([P, NQ], F32, tag="big", bufs=5)
                nc.tensor.matmul(
                    su[0:2, :], lhsT=ones2_sb[:], rhs=sq[:],
                    start=True, stop=True,
                )
                # 1/(sqrt(x)) = exp(-0.5*ln(x)): stays in the Exp/Ln ACT
                # table set; [2, 512] rows are partition-parallel on ACT
                lnr = sb.tile([2, NQ], F32, tag="lnr", bufs=3)
                nc.scalar.activation(lnr[:], su[0:2, :], AF.Ln)
                nrm = sb.tile([2, NQ], F32, tag="nrm", bufs=3)
                nc.scalar.activation(nrm[:], lnr[:], AF.Exp, scale=-0.5)
                nrow = nrow_d.ap()[ridx]
                nc.scalar.dma_start(nrow, nrm[:])
                rb = sb.tile([P, NQ], F32, tag="rb", bufs=3)
                nc.scalar.dma_start(
                    rb[0:64, :], nrow[0:1].to_broadcast((64, NQ))
                )
                nc.scalar.dma_start(
                    rb[64:128, :], nrow[1:2].to_broadcast((64, NQ))
                )
                nc.vector.tensor_mul(out_t[:], pp[:], rb[:])

            for c in range(NQC):
                for jt in range(2):
                    qt_t[(jt, c)] = sb.tile(
                        [P, NQ], F32R, tag=f"qt{jt}{c}", name=f"qt{jt}{c}"
                    )
                    proj_chunk(wq8_sb, jt, c, qt_t[(jt, c)], 4 * jt + c)
                for jt in range(2):
                    kt_t[(jt, c)] = sb.tile(
                        [P, NQ], F32R, tag=f"kt{jt}{c}", name=f"kt{jt}{c}"
                    )
                    proj_chunk(wk8_sb, jt, c, kt_t[(jt, c)], 8 + 4 * jt + c)
                for tt in range(4 * c, 4 * c + 4):
                    pp = ps.tile([P, J], F32, tag="big", bufs=5)
                    for i in range(ID):
                        nc.tensor.matmul(
                            pp[:],
                            lhsT=xb_sb[c][:, i, bass.ts(tt % 4, P)],
                            rhs=wvb_sb[:, i, :],
                            start=(i == 0),
                            stop=(i == ID - 1),
                        )
                    nc.vector.tensor_copy(
                        v4[:, tt, :, 0:DH],
                        pp[:].rearrange("p (h x) -> p h x", x=DH),
                    )

            # ---- attention + pipelined output projection ----
            # vhat/ysb/rbo reuse xt-grid slots (xt dead after projections)
            vhat_q = {
                (jt, qc): sb.tile(
                    [P, NQ], F32R, tag=f"x{4 * jt + qc}c3", name=f"vhat{jt}_{qc}"
                )
                for jt in range(2)
                for qc in range(NQC)
            }
            CH = 3
            for qc in range(NQC):
                nkt = 4 * qc + 4
                for hp in range(2):
                    ots = [
                        ps.tile([P, NQ], F32, tag="ot", bufs=3, name=f"ot{i}")
                        for i in range(2)
                    ]
                    for c0 in range(0, nkt, CH):
                        kts = range(c0, min(c0 + CH, nkt))
                        pts = {}
                        sts = {}
                        for kt in kts:
                            for h01 in range(2):
                                hsl = slice(64 * h01, 64 * h01 + 64)
                                st = ps.tile([P, NQ], F32, tag="big", bufs=5)
                                nc.tensor.matmul(
                                    st[:],
                                    lhsT=kt_t[(hp, kt // 4)][
                                        hsl, bass.ts(kt % 4, P)
                                    ],
                                    rhs=qt_t[(hp, qc)][hsl, :],
                                    start=True,
                                    stop=True,
                                )
                                sts[(kt, h01)] = st
                        for kt in kts:
                            dj = kt - 4 * qc  # >=0: diagonal-crossing tile
                            for h01 in range(2):
                                pt = sb.tile([P, NQ], F32R, tag="pt", bufs=8)
                                if dj >= 1:
                                    # cols < 128*dj fully causal-masked
                                    nc.vector.tensor_scalar_mul(
                                        pt[:, 0 : P * dj],
                                        sts[(kt, h01)][:, 0 : P * dj],
                                        0.0,
                                    )
                                    nc.scalar.activation(
                                        pt[:, P * dj :],
                                        sts[(kt, h01)][:, P * dj :],
                                        AF.Exp,
                                        scale=1.0 / math.sqrt(DH),
                                    )
                                else:
                                    nc.scalar.activation(
                                        pt[:], sts[(kt, h01)][:], AF.Exp,
                                        scale=1.0 / math.sqrt(DH),
                                    )
                                if dj >= 0:
                                    blk = slice(P * dj, P * dj + P)
                                    nc.vector.tensor_mul(
                                        pt[:, blk], pt[:, blk], mask_sb[:]
                                    )
                                pts[(kt, h01)] = pt
                        for kt in kts:
                            for h01 in range(2):
                                h = 2 * hp + h01
                                nc.tensor.matmul(
                                    ots[h01][0:VW, :],
                                    lhsT=v_sb[:, kt, VW * h : VW * h + VW],
                                    rhs=pts[(kt, h01)][:],
                                    start=(kt == 0),
                                    stop=(kt == nkt - 1),
                                )
                    for h01 in range(2):
                        # 1/x = exp(-ln(x)) on ACT (same table set as exp)
                        dln = sb.tile([1, NQ], F32, tag="dln", bufs=2)
                        nc.scalar.activation(
                            dln[0:1, :], ots[h01][DH : DH + 1, :], AF.Ln
                        )
                        den = sb.tile([1, NQ], F32, tag="den", bufs=2)
                        nc.scalar.activation(
                            den[0:1, :], dln[0:1, :], AF.Exp, scale=-1.0
                        )
                        didx = 8 * hp + 2 * qc + h01
                        drow = den_d.ap()[didx : didx + 1, :]
                        nc.scalar.dma_start(drow, den[0:1, :])
                        rbo = sb.tile([P, NQ], F32, tag=f"x{6 + h01}c2", bufs=1)
                        nc.scalar.dma_start(
                            rbo[0:64, :], drow.to_broadcast((64, NQ))
                        )
                        nc.vector.tensor_mul(
                            vhat_q[(hp, qc)][64 * h01 : 64 * h01 + 64, :],
                            ots[h01][0:DH, :],
                            rbo[0:64, :],
                        )

                # output projection for this q-chunk + chunk ReduceScatter
                ypv = yparts[qc].ap()
                for t4 in range(4):
                    for mc in range(2):
                        msl = bass.ts(mc, NQ)
                        yp = ps.tile([P, NQ], F32, tag="big", bufs=5)
                        for jt in range(2):
                            nc.tensor.matmul(
                                yp[:],
                                lhsT=vhat_q[(jt, qc)][:, bass.ts(t4, P)],
                                rhs=wo_sb[:, jt, msl],
                                start=(jt == 0),
                                stop=(jt == 1),
                            )
                        ysb = sb.tile(
                            [P, NQ], BF16, tag=f"x{(2 * t4 + mc) % 6}c2", bufs=1
                        )
                        nc.vector.tensor_add(ysb[:], yp[:], bias_sb[:, msl])
                        nc.sync.dma_start(ypv[t4, mc], ysb[:])

                cc = nc.gpsimd.collective_compute(
                    "ReduceScatter",
                    mybir.AluOpType.add,
                    replica_groups=[[0, 1, 2, 3], [4, 5, 6, 7]],
                    ins=[yparts[qc].ap()],
                    outs=[yrss[qc].ap()],
                )
                outdma = nc.sync.dma_start(y_ext[qc], yrss[qc].ap())
                add_dep(outdma.ins, cc.ins, sync=True, reason="out after rs")

    _split_excess_waits(nc)
    return nc


_NC = None

# test-harness hooks: set TRACE=True before calling kernel() to capture a
# perfetto trace; results land in LAST_RESULTS
TRACE = False
TRACE_DIR = None
LAST_RESULTS = None


def _get_nc():
    global _NC
    if _NC is None:
        _NC = _build()
    return _NC


def _make_mask():
    r = np.arange(P)[:, None]
    c = np.arange(P)[None, :]
    return (r <= c).astype(np.float32)


def kernel(X, Wq, Wk, Wv, Wo, bo):
    X = np.asarray(X, dtype=np.float32)
    Wq = np.asarray(Wq, dtype=np.float32)
    Wk = np.asarray(Wk, dtype=np.float32)
    Wv = np.asarray(Wv, dtype=np.float32)
    Wo = np.asarray(Wo, dtype=np.float32)
    bo = np.asarray(bo, dtype=np.float32)

    nc = _get_nc()
    mask = _make_mask()
    ones = np.ones((P, 1), np.float32)
    ones2 = np.zeros((P, 2), np.float32)
    ones2[0:64, 0] = 1.0
    ones2[64:128, 1] = 1.0
    ones2 = ones2.astype(ml_dtypes.bfloat16)
    bias4 = (bo * 0.25).astype(np.float32)
    x8s, xbs = [], []
    for b in range(B):
        XT = X[b].T  # [1024 d, 2048 n]
        # x8[dr, qc, p, t, n] = X[b, 512qc+n, 256dr+128t+p], fp8
        x8s.append(
            np.ascontiguousarray(
                XT.reshape(NDR, 2, P, NQC, NQ).transpose(0, 3, 2, 1, 4)
            ).astype(ml_dtypes.float8_e4m3fn)
        )
        # xb[qc, i, p, n] = X[b, 512qc+n, 128i+p], bf16
        xbs.append(
            np.ascontiguousarray(
                XT.reshape(ID, P, NQC, NQ).transpose(2, 0, 1, 3)
            ).astype(ml_dtypes.bfloat16)
        )

    def w8slice(W, jsl):
        # [1024, 256] -> [NDR, 128, 2, 256] DoubleRow pairs, scaled fp8
        return np.ascontiguousarray(
            (W[:, jsl] * WSCALE).reshape(NDR, 2, P, J).transpose(0, 2, 1, 3)
        ).astype(ml_dtypes.float8_e4m3fn)

    in_maps = []
    for c in range(8):
        b, g = c // 4, c % 4
        jsl = slice(g * J, (g + 1) * J)
        in_maps.append(
            {
                "x8": x8s[b],
                "xb": xbs[b],
                "wq8": w8slice(Wq, jsl),
                "wk8": w8slice(Wk, jsl),
                "wvb": np.ascontiguousarray(
                    Wv[:, jsl].reshape(ID, P, J)
                ).astype(ml_dtypes.bfloat16),
                "wo": np.ascontiguousarray(
                    Wo[jsl, :].reshape(2, P, D).transpose(1, 0, 2)
                ),
                "bias4": bias4,
                "maskd": mask,
                "onesd": ones,
                "ones2d": ones2,
            }
        )

    global LAST_RESULTS
    if TRACE:
        res = run_bass_kernel_spmd(
            nc, in_maps, list(range(8)), trace=True, tmpdir=TRACE_DIR
        )
    else:
        res = run_bass_kernel_spmd(nc, in_maps, list(range(8)))
    LAST_RESULTS = res
    out = np.empty((B, N, D), np.float32)
    for c in range(8):
        b, r = c // 4, c % 4
        yc = np.asarray(res.results[c]["y"]).astype(np.float32)
        for qc in range(NQC):
            rows = slice(NQ * qc + P * r, NQ * qc + P * r + P)
            out[b, rows, 0:NQ] = yc[qc, 0]
            out[b, rows, NQ:D] = yc[qc, 1]
    return out

